# revision 23
# baseline (speedup 1.0000x reference)
"""Trainium2 Bass kernel: self-attention block with interleaved RoPE.

Reference computation (per batch b):
    qkv = x @ qkv_w.T + qkv_b            # [L, 3C]
    q,k,v per head (H=16, D=64); q,k get interleaved RoPE
    attn = softmax(scale * q @ k.T) @ v  # per head
    out  = concat_heads(attn) @ proj_w.T + proj_b

Sharding: 8 cores = 4 batches x 2 query-halves. Each core computes K,V for
the full sequence of its batch (duplicated across the half-pair) and Q for
its own 1024 queries -> disjoint output slices, no collectives.

Per-core layout trick: the host permutes the L axis of x^T (and cos/sin) so
this core's queries are always columns 0:1024 -> one SPMD graph for all
cores. Key order is permuted consistently everywhere (softmax is
order-invariant over keys).

On-chip dataflow (all "transposed" so no PE transposes are needed):
  K'^T/Q'^T [d, L] tiles  <- matmul(lhsT=W^T slab, rhs=x^T) + RoPE on DVE
     (rotate_half realized as 32-partition block-swap DMAs)
  S^T [k,q] = matmul(lhsT=K'^T, rhs=Q'^T) for BOTH heads of a pair,
     row-group packed (sel=0 -> array rows 0:64, sel=1 -> rows 64:128,
     auto tile_position) -> the two score MMs run CONCURRENTLY.
  P^T = exp(scale*S^T) on ACT (no max subtraction: scores ~N(0,1), exp
     cannot overflow in f32)
  AV^T [65,q] = matmul(lhsT=[V | ones], rhs=P^T)   row 64 = softmax denom
  normalize: PSUM evacuate on ACT (Copy shares the exp table set), then
     the [1,1024] denominator row is DMA-reshaped to [16,64] so the DVE
     reciprocal isn't lane-starved, reshaped+cast back to bf16 by one
     gpsimd software-DGE DMA, broadcast with a cheap bf16 ones-matmul,
     and multiplied into stk on DVE.
  proj: stacked head pairs -> matmul over 8 contraction chunks; PSUM->SBUF
     evacuation adds the (pre-replicated) bias when biases are nonzero.

Schedule: ONE flat stream of 256 attention (pair,j,t) steps paced by ACT
(exp ~1.0us/step). ALL generation work (K/Q/V matmul chunks, split into
4-MM half-units ~0.86us each) sits in a deadline-sorted queue; ~one unit is
popped per attention step so the PE never idles (HAM stays at K=8/8) and
there is no un-overlapped gen prologue. need() force-pops units whose
output an attention step is about to consume. Normalize finishers and proj
tiles enter the same queue once their inputs exist; the final tail
interleaves the last normalize chain with proj head-chunks.

The common case (all-zero qkv/proj biases, as in this problem's inputs)
builds a graph with no bias adds at all and 1.5MB less prologue DMA; a
with-bias graph is built instead if kernel() sees nonzero biases.
"""

import numpy as np
import ml_dtypes

import concourse.bass as bass
import concourse.mybir as mybir
from concourse.tile import TileContext

F32 = mybir.dt.float32
BF16 = mybir.dt.bfloat16
AOP = mybir.AluOpType
AFT = mybir.ActivationFunctionType

B, L, C = 4, 2048, 1024
H, D = 16, 64
LQ = L // 2            # queries per core
NPAIR = H // 2         # 8 head pairs
NG = 4                 # head groups
GH = H // NG           # 4 heads per group
GP = GH // 2           # 2 pairs per group
CCH = C // 128         # 8 contraction chunks
LT = L // 128          # 16 key tiles
QT = LQ // 128         # 8 query row-tiles
SCALE = float(D) ** -0.5


def build_nc(with_bias=False):
    nc = bass.Bass()
    xT = nc.declare_dram_parameter("xT", [C, L], BF16, isOutput=False)
    wcat = nc.declare_dram_parameter("wcat", [2 * C // 128, 128, C], BF16, isOutput=False)
    pT = nc.declare_dram_parameter("pT", [C, C], BF16, isOutput=False)
    cosP = nc.declare_dram_parameter("cosP", [128, L], BF16, isOutput=False)
    sinP = nc.declare_dram_parameter("sinP", [128, L], BF16, isOutput=False)
    if with_bias:
        pb = nc.declare_dram_parameter("pb", [128, C], F32, isOutput=False)
        bK = nc.declare_dram_parameter("bK", [128, NPAIR], F32, isOutput=False)
        bQ = nc.declare_dram_parameter("bQ", [128, NPAIR], F32, isOutput=False)
        vb = nc.declare_dram_parameter("vb", [128, C], F32, isOutput=False)
    ones64 = nc.declare_dram_parameter("ones64", [1, 64], BF16, isOutput=False)
    vcat = nc.declare_dram_parameter("vcat", [2, 128, 4 * C], BF16, isOutput=False)
    out = nc.declare_dram_parameter("out", [LQ, C], F32, isOutput=True)

    with TileContext(nc) as tc:
        with (
            tc.tile_pool(name="persist", bufs=1) as P1,
            tc.tile_pool(name="wpool", bufs=2) as WP,
            tc.tile_pool(name="vwpool", bufs=2) as VW,
            tc.tile_pool(name="ktpool", bufs=2) as KTP,
            tc.tile_pool(name="work", bufs=2) as WK,
            tc.tile_pool(name="osbpool", bufs=4) as OSB,
            tc.tile_pool(name="espool", bufs=3) as ESP,
            tc.tile_pool(name="vtpool", bufs=2) as VTP,
            tc.tile_pool(name="pgen", bufs=2, space="PSUM") as PGEN,
            tc.tile_pool(name="pss", bufs=2, space="PSUM") as PSS,
            tc.tile_pool(name="pav", bufs=1, space="PSUM") as PAV,
        ):
            # ---- PE warmup: ~7us of tiny matmuls on a memset tile so the
            # HAM clock-gate holds K=8/8 until the first x/weight DMAs land
            # (~14us); sized to end just before so real MMs aren't queued
            # behind the spin.
            wrm = P1.tile([128, 64], BF16, name="wrm", tag="wrm")
            nc.vector.memset(wrm[:, :], 0.125)
            pswm = PGEN.tile([64, 64], F32, name="pswm", tag="gen")
            for _ in range(90):
                nc.tensor.matmul(pswm[:, :], wrm[:, 0:64], wrm[:, 0:64],
                                 start=True, stop=True)

            # ---- x^T: first quarter split in half (the very first K/Q gen
            # chunk only needs contraction rows 0:512), rest by quarters.
            xtile = P1.tile([128, CCH * L], BF16, name="xtile", tag="xtile")
            xt = [xtile[:, i * L:(i + 1) * L] for i in range(CCH)]
            xv3 = xtile.rearrange("p (k l) -> p k l", l=L)
            xs3 = xT.rearrange("(k p) l -> p k l", p=128)
            nc.sync.dma_start(out=xv3[:, 0:4, 0:512], in_=xs3[:, 0:4, 0:512])
            nc.sync.dma_start(out=xv3[:, 4:8, 0:512], in_=xs3[:, 4:8, 0:512])
            for lh in range(1, 4):
                lsl = slice(lh * (L // 4), (lh + 1) * (L // 4))
                nc.sync.dma_start(out=xv3[:, :, lsl], in_=xs3[:, :, lsl])
            touch_n = [0]

            def touch(t):
                # tiny DVE read so later DVE ops don't each carry this
                # tile's DMA-queue semaphore wait (walrus wait-count limit)
                sc = P1.tile([1, 1], F32, name=f"scr{touch_n[0]}",
                             tag=f"scr{touch_n[0]}")
                touch_n[0] += 1
                nc.vector.tensor_copy(sc[0:1, 0:1], t[0:1, 0:1])

            # persistent small loads on the SCALAR engine's DMA queue so
            # they don't serialize behind the 4MB x load on sync
            cos_sb = P1.tile([128, L], BF16, name="cos_sb", tag="cos_sb")
            nc.scalar.dma_start(out=cos_sb[:, :], in_=cosP[:, :])
            touch(cos_sb)
            sin_sb = P1.tile([128, L], BF16, name="sin_sb", tag="sin_sb")
            nc.scalar.dma_start(out=sin_sb[:, :], in_=sinP[:, :])
            touch(sin_sb)
            bias_sb = {}
            if with_bias:
                for nm, prm in (("bK", bK), ("bQ", bQ)):
                    t = P1.tile([128, NPAIR], F32, name=f"{nm}_sb",
                                tag=f"{nm}_sb")
                    nc.scalar.dma_start(out=t[:, :], in_=prm[:, :])
                    touch(t)
                    bias_sb[nm] = t
                vb_sb = P1.tile([128, C], F32, name="vb_sb", tag="vb_sb")
                nc.scalar.dma_start(out=vb_sb[:, :], in_=vb[:, :])
                touch(vb_sb)
            ones_sb = P1.tile([1, 64], BF16, name="ones_sb", tag="ones_sb")
            nc.scalar.dma_start(out=ones_sb[:, :], in_=ones64[:, :])
            # proj weights (and bias) are needed only from step ~190 on;
            # loaded via deferred queue units to keep prologue DMA light.
            pt = [P1.tile([128, C], BF16, name=f"pt{i}", tag=f"pt{i}")
                  for i in range(CCH)]
            pb_sb = (P1.tile([128, C], F32, name="pb_sb", tag="pb_sb")
                     if with_bias else None)
            stk = []
            for i in range(NPAIR):
                stk.append(P1.tile([128, LQ], BF16, name=f"stk{i}", tag=f"stk{i}"))

            kt_of = {}   # pair -> ktile
            qt_of = {}   # pair -> qtile
            wt_of = {}   # (pair, kind) -> weight slab
            vsl_of = {}  # sg -> V weight slab
            vt_sg = {}   # sg -> list of vt tiles
            psm_of = {}  # (pair, kind, j) -> open gen psum
            psv_of = {}  # (sg, t) -> open v psum
            pj_of = {}   # (qi, jn) -> open proj psum

            # ---------- work units ----------
            def mk_slab(p):
                def f():
                    lp = p % GP
                    kt_of[p] = KTP.tile([128, L], BF16, name=f"kt{lp}",
                                        tag=f"ktile{lp}")
                    qt_of[p] = KTP.tile([128, LQ], BF16, name=f"qt{lp}",
                                        tag=f"qtile{lp}")
                    for kind, base in ((0, 0), (1, C)):
                        wt_ = WP.tile([128, C], BF16, name="wt_",
                                      tag=f"wt{lp}_{kind}")
                        nc.gpsimd.dma_start(
                            out=wt_[:, :], in_=wcat[base // 128 + p])
                        wt_of[(p, kind)] = wt_
                return f

            def mk_vsl(sg):
                def f():
                    vt_sg[sg] = [None] * LT
                    vsl = VW.tile([128, 4 * C], BF16, name="vsl", tag="vsl")
                    nc.gpsimd.dma_start(out=vsl[:, :], in_=vcat[sg])
                    vsl_of[sg] = vsl
                return f

            def mk_ptload(i):
                def f():
                    nc.sync.dma_start(out=pt[i][:, :],
                                      in_=pT[i * 128:(i + 1) * 128, :])
                    if with_bias and i == 0:
                        nc.scalar.dma_start(out=pb_sb[:, :], in_=pb[:, :])
                return f

            def mk_kq(p, kind, j, h):
                def f():
                    key = (p, kind, j)
                    if h == 0:
                        psm_of[key] = PGEN.tile([128, 512], F32, name="psm",
                                                tag="gen")
                    psm = psm_of[key]
                    wt_ = wt_of[(p, kind)]
                    jsl = slice(j * 512, (j + 1) * 512)
                    for kc in range(4 * h, 4 * h + 4):
                        nc.tensor.matmul(
                            psm[:, :],
                            wt_[:, kc * 128:(kc + 1) * 128],
                            xt[kc][:, jsl],
                            start=(kc == 0), stop=(kc == CCH - 1),
                        )
                    if h == 0:
                        return
                    # RoPE: raw (biased) values in bf16, rotate-half as
                    # 32-partition block-swap DMAs (host de-interleaved the
                    # d order: evens then odds; sinP carries the sign)
                    dst = kt_of[p] if kind == 0 else qt_of[p]
                    kr = WK.tile([128, 512], BF16, name="kr", tag="kr")
                    if with_bias:
                        bmain = "bK" if kind == 0 else "bQ"
                        nc.vector.tensor_scalar_add(
                            kr[:, :], psm[:, :], bias_sb[bmain][:, p:p + 1])
                    else:
                        nc.vector.tensor_copy(kr[:, :], psm[:, :])
                    krs = WK.tile([128, 512], BF16, name="krs", tag="krs")
                    for bb in range(4):
                        so = (bb ^ 1) * 32
                        nc.sync.dma_start(
                            out=krs[bb * 32:(bb + 1) * 32, :],
                            in_=kr[so:so + 32, :],
                        )
                    if with_bias:
                        nc.vector.scalar_tensor_tensor(
                            out=dst[:, jsl], in0=psm[:, :],
                            scalar=bias_sb[bmain][:, p:p + 1],
                            in1=cos_sb[:, jsl], op0=AOP.add, op1=AOP.mult,
                        )
                    else:
                        nc.vector.tensor_tensor(
                            out=dst[:, jsl], in0=psm[:, :],
                            in1=cos_sb[:, jsl], op=AOP.mult,
                        )
                    tmp = WK.tile([128, 512], BF16, name="tmp", tag="tmp")
                    nc.vector.tensor_tensor(
                        out=tmp[:, :], in0=krs[:, :], in1=sin_sb[:, jsl],
                        op=AOP.mult,
                    )
                    nc.vector.tensor_tensor(
                        out=dst[:, jsl], in0=dst[:, jsl], in1=tmp[:, :],
                        op=AOP.add,
                    )
                return f

            def mk_v(sg, t, h):
                def f():
                    key = (sg, t)
                    if h == 0:
                        vt_sg[sg][t] = VTP.tile([128, 2 * GH * 65], BF16,
                                                name="vt", tag=f"vt{t}")
                        psv_of[key] = PGEN.tile([128, 512], F32, name="psv",
                                                tag="gen")
                    psv = psv_of[key]
                    vsl = vsl_of[sg]
                    for kc in range(4 * h, 4 * h + 4):
                        nc.tensor.matmul(
                            psv[:, :],
                            xt[kc][:, t * 128:(t + 1) * 128],
                            vsl[:, kc * 512:(kc + 1) * 512],
                            start=(kc == 0), stop=(kc == CCH - 1),
                        )
                    if h == 0:
                        return
                    vt = vt_sg[sg][t]
                    vt3 = vt.rearrange("p (a s) -> p a s", s=65)
                    if with_bias:
                        nc.vector.tensor_tensor(
                            out=vt3[:, :, 0:64],
                            in0=psv[:, :].rearrange("p (a d) -> p a d", d=64),
                            in1=vb_sb[:, sg * 512:(sg + 1) * 512]
                            .rearrange("p (a d) -> p a d", d=64),
                            op=AOP.add,
                        )
                    else:
                        # evacuate on ACT (Copy, same table set as Exp):
                        # keeps the DVE free of psv-consumer backlog that
                        # otherwise stalls gen-psum buffer reuse on the PE
                        nc.scalar.copy(
                            vt3[:, :, 0:64],
                            psv[:, :].rearrange("p (a d) -> p a d", d=64),
                        )
                    nc.vector.memset(vt3[:, :, 64:65], 1.0)
                return f

            def mk_proj(qi, jn, h, ptag="gen"):
                # h0: contraction chunks 0..5 (stk of pairs 0..5, available
                # well before the tail); h1: chunks 6,7 + evacuate. At the
                # tail, half the tiles borrow the (by then idle) ss psum
                # bufs so 4 accumulations can be open at once.
                def f():
                    key = (qi, jn)
                    if h == 0:
                        pool = PGEN if ptag == "gen" else PSS
                        pj_of[key] = pool.tile([128, 512], F32, name="pj",
                                               tag=ptag)
                    pj = pj_of[key]
                    rng = range(0, 6) if h == 0 else range(6, CCH)
                    for cp in rng:
                        nc.tensor.matmul(
                            pj[:, :],
                            stk[cp][:, qi * 128:(qi + 1) * 128],
                            pt[cp][:, jn * 512:(jn + 1) * 512],
                            start=(cp == 0), stop=(cp == CCH - 1),
                        )
                    if h == 0:
                        return
                    osl = OSB.tile([128, 512], F32, name="osb", tag="osb")
                    if with_bias:
                        nc.vector.tensor_tensor(
                            out=osl[:, :], in0=pj[:, :],
                            in1=pb_sb[:, jn * 512:(jn + 1) * 512], op=AOP.add,
                        )
                    else:
                        nc.vector.tensor_copy(osl[:, :], pj[:, :])
                    nc.sync.dma_start(
                        out=out[qi * 128:(qi + 1) * 128,
                                jn * 512:(jn + 1) * 512],
                        in_=osl[:, :],
                    )
                return f

            # ---------- queue: (label, light, is_h0, fn) ----------
            queue = []
            emitted = set()

            def build_units():
                units = []  # (deadline, seq, label, light, is_h0, fn)
                seq = [0]

                def add(dl, label, light, is_h0, fn):
                    units.append((dl, seq[0], label, light, is_h0, fn))
                    seq[0] += 1

                for p in range(8):
                    base = 32 * p
                    add(base - 6, f"slab{p}", True, False, mk_slab(p))
                    if p % 4 == 0:
                        sg = p // 4
                        add(base - 5.5, f"vsl{sg}", True, False, mk_vsl(sg))
                    # K/Q units get ~4 steps of slack before first use: the
                    # RoPE tail (kr copy -> 4 swap DMAs -> 2 DVE ops) is
                    # ~5us of latency that otherwise blocks the pair's
                    # first score matmul at the boundary
                    for jk in range(4):
                        for h in range(2):
                            add(base + 4 * jk - 4.5 + 0.01 * h,
                                f"K{p}_{jk}" if h else f"K{p}_{jk}h0",
                                False, h == 0, mk_kq(p, 0, jk, h))
                    for jq in range(2):
                        for h in range(2):
                            add(base + 16 * jq - 4.4 + 0.01 * h,
                                f"Q{p}_{jq}" if h else f"Q{p}_{jq}h0",
                                False, h == 0, mk_kq(p, 1, jq, h))
                for sg in range(2):
                    for t in range(LT):
                        for h in range(2):
                            add(128 * sg + t - 2.3 + 0.01 * h,
                                f"V{sg}_{t}" if h else f"V{sg}_{t}h0",
                                False, h == 0, mk_v(sg, t, h))
                for i in range(CCH):
                    add(150 + 2 * i, f"pt{i}", True, False, mk_ptload(i))
                units.sort(key=lambda u: (u[0], u[1]))
                return [(lbl, light, h0, fn) for _, _, lbl, light, h0, fn in units]

            queue.extend(build_units())

            def pop_one():
                lbl, light, h0, fn = queue.pop(0)
                fn()
                emitted.add(lbl)
                return light

            def pop_some():
                while queue:
                    if not pop_one():
                        break

            def need(lbl):
                while lbl not in emitted:
                    assert queue, f"need({lbl}) but queue empty"
                    pop_one()

            def insert_unit(unit, min_pos):
                # insert at the first group boundary (is_h0 or light) at or
                # after min_pos so an h0/h1 psum pair is never split
                pos = min(min_pos, len(queue))
                while pos < len(queue) and not (queue[pos][1] or queue[pos][2]):
                    pos += 1
                queue.insert(pos, unit)

            # ---------- normalize finisher ----------
            def fin_pre(p, j, av, tail=False):
                # PSUM->SBUF evacuation on ACT (Copy shares the exp table
                # set, no reload): frees the single av bank ~1.2us after the
                # last AV matmul regardless of DVE backlog, so the next
                # (pair,j)'s AV t0 doesn't stall the PE.
                avc = WK.tile([65, 1024], F32, name="avc", tag="avc")
                nc.scalar.copy(avc[:, :], av[:, :])
                # reciprocal of the [1,1024] denominator row is DVE
                # lane-starved (6.6us). DMA-reshape to [16,64] (64 elems
                # per lane), reciprocal there (~0.5us), then one gpsimd
                # software-DGE DMA reshapes back AND casts f32->bf16.
                # (Routing these DMAs via the sync queue measured slower:
                # they contend with the rotate-half swap DMAs there.)
                rds = WK.tile([16, 64], F32, name="rds", tag="rds")
                rdr = WK.tile([16, 64], F32, name="rdr", tag="rdr")
                rdb = WK.tile([1, 1024], BF16, name="rdb", tag="rdb")
                if tail:
                    # latency matters only here: hardware-DGE DMAs on the
                    # (idle at tail) scalar queue + a cheap [16,64] DVE cast
                    nc.scalar.dma_start(out=rds[:, :], in_=avc[64:65, :])
                    nc.vector.reciprocal(rdr[:, :], rds[:, :])
                    rdc = WK.tile([16, 64], BF16, name="rdc", tag="rdc")
                    nc.vector.tensor_copy(rdc[:, :], rdr[:, :])
                    nc.scalar.dma_start(out=rdb[0:1, :], in_=rdc[:, :])
                else:
                    nc.gpsimd.dma_start(out=rds[:, :], in_=avc[64:65, :])
                    nc.vector.reciprocal(rdr[:, :], rds[:, :])
                    nc.gpsimd.dma_start(out=rdb[0:1, :], in_=rdr[:, :])

                jsl = slice(j * 512, (j + 1) * 512)

                def post():
                    # the tail finisher sits between two OPEN proj psum
                    # accumulations; allocating from the (now idle) PAV
                    # bank avoids a "gen"-tag reuse cycle there.
                    if tail:
                        bcw = PAV.tile([64, 1024], F32, name="bcw", tag="av")
                        bcs = [bcw[:, 0:512], bcw[:, 512:1024]]
                    else:
                        bcs = [PGEN.tile([64, 512], F32, name="bc", tag="gen")
                               for _ in range(2)]
                    for sel in range(2):
                        bc = bcs[sel]
                        nc.tensor.matmul(
                            bc[:, :], ones_sb[:, :],
                            rdb[0:1, sel * 512:(sel + 1) * 512],
                            start=True, stop=True,
                        )
                        nc.vector.tensor_tensor(
                            out=stk[p][sel * 64:(sel + 1) * 64, jsl],
                            in0=avc[0:64, sel * 512:(sel + 1) * 512],
                            in1=bc[:, :], op=AOP.mult,
                        )
                return post

            # ---------- main schedule ----------
            pj_order = []
            for g in range(NG - 1):
                for lp in range(GP):
                    for j in range(2):
                        pj_order.append((2 * g + lp, j))
            for j in range(2):
                for lp in range(GP):
                    pj_order.append((6 + lp, j))

            def pop_allowed(p, j, t):
                if (p, j) in ((6, 1), (7, 1)):
                    return 5 <= t <= 14 and t != 10
                return t not in (0, 5, 10, 15)

            for p, j in pj_order:
                sg = p // 4
                voff = ((p // 2) % 2) * GH * 65
                lp = p % GP
                jsl = slice(j * 512, (j + 1) * 512)
                need(f"Q{p}_{j}")
                av = PAV.tile([65, 1024], F32, name="av", tag="av")
                for t in range(LT):
                    need(f"K{p}_{t // 4}")
                    need(f"V{sg}_{t}")
                    if pop_allowed(p, j, t) and queue:
                        pop_some()
                    ss = PSS.tile([128, 1024], F32, name="ss", tag="ss")
                    for sel in range(2):
                        nc.tensor.matmul(
                            ss[:, sel * 512:(sel + 1) * 512],
                            kt_of[p][sel * 64:(sel + 1) * 64,
                                     t * 128:(t + 1) * 128],
                            qt_of[p][sel * 64:(sel + 1) * 64, jsl],
                            start=True, stop=True,
                        )
                    es = ESP.tile([128, 1024], BF16, name="es", tag="es")
                    nc.scalar.activation(es[:, :], ss[:, :], AFT.Exp,
                                         scale=SCALE)
                    for sel in range(2):
                        lh = lp * 2 + sel
                        nc.tensor.matmul(
                            av[0:65, sel * 512:(sel + 1) * 512],
                            vt_sg[sg][t][:, voff + lh * 65:voff + (lh + 1) * 65],
                            es[:, sel * 512:(sel + 1) * 512],
                            start=(t == 0), stop=(t == LT - 1),
                        )
                post = fin_pre(p, j, av, tail=(p, j) == (7, 1))
                if (p, j) == (7, 1):
                    # tail: FOUR open proj accumulations (alternating
                    # gen/ss psum tags) cover the last normalize chain,
                    # then the finisher, then h1 units (which read
                    # stk[6],stk[7]) pipelined with the remaining h0s
                    tiles = [(qi, jn) for qi in range(QT // 2, QT)
                             for jn in range(2)]
                    tag = {t: ("gen" if i % 2 == 0 else "ss")
                           for i, t in enumerate(tiles)}
                    hu = {(qi, jn, h): (f"pj{qi}_{jn}h{h}", False, h == 0,
                                        mk_proj(qi, jn, h, tag[(qi, jn)]))
                          for qi, jn in tiles for h in range(2)}
                    for i in range(4):
                        queue.append(hu[tiles[i] + (0,)])
                    queue.append((f"fin{p}_{j}", True, False, post))
                    for i, (qi, jn) in enumerate(tiles):
                        queue.append(hu[(qi, jn, 1)])
                        if i + 4 < len(tiles):
                            queue.append(hu[tiles[i + 4] + (0,)])
                else:
                    insert_unit((f"fin{p}_{j}", True, False, post), 3)
                if (p, j) == (7, 0):
                    for qi in range(QT // 2):
                        for jn in range(2):
                            for h in range(2):
                                queue.append((f"pj{qi}_{jn}h{h}", False,
                                              h == 0, mk_proj(qi, jn, h)))
            while queue:
                pop_one()
    return nc


_CACHE = {}

# walrus in this toolchain enforces small per-instruction sync-wait budgets
# (DMACopy/TensorCopy: 1, most compute: 2). Tile emits more on a few
# instructions, so split the excess into standalone EventSemaphore
# wait-carriers on the same engine (the raw-bass wait_ge pattern).
_WAIT_BUDGET = {"DMACopy": 1, "TensorCopy": 1, "Reciprocal": 1, "Memset": 1,
                "Iota": 1, "FindIndex8": 1}
_DEFAULT_BUDGET = 1


def _split_waits(bir_bytes):
    import json
    bir = json.loads(bir_bytes)
    ctr = 0
    for fn in bir["functions"]:
        for blk in fn["blocks"]:
            insts = blk.get("instructions")
            if not insts:
                continue
            out = []
            for inst in insts:
                si = inst.get("sync_info")
                if si and si.get("on_wait"):
                    waits = si["on_wait"]
                    b = _WAIT_BUDGET.get(inst.get("opcode"), _DEFAULT_BUDGET)
                    if len(waits) > b:
                        excess, keep = waits[:-b], waits[-b:]
                        for w in excess:
                            ctr += 1
                            out.append({
                                "debug": inst.get("debug", 0),
                                "engine": inst["engine"],
                                "ins": [], "outs": [],
                                "name": f"wfix{ctr}",
                                "opcode": "EventSemaphore",
                                "sync_info": {"on_update": [], "on_wait": [w]},
                            })
                        si["on_wait"] = keep
                out.append(inst)
            blk["instructions"] = out
    return json.dumps(bir).encode()


def _get_nc(with_bias=False):
    key = f"nc{int(with_bias)}"
    if key not in _CACHE:
        nc = build_nc(with_bias)
        fixed = _split_waits(nc.to_json_bytes())
        nc.to_json_bytes = lambda fixed=fixed: fixed
        _CACHE[key] = nc
    return _CACHE[key]


def make_in_maps(x, cos_emb, sin_emb, qkv_w, qkv_b, proj_w, proj_b,
                 with_bias=None):
    f32 = np.float32
    x = np.asarray(x, f32)
    qkv_w = np.asarray(qkv_w, f32)
    qkv_b = np.asarray(qkv_b, f32)
    proj_w = np.asarray(proj_w, f32)
    proj_b = np.asarray(proj_b, f32)
    cos_emb = np.asarray(cos_emb, f32)
    sin_emb = np.asarray(sin_emb, f32)
    if with_bias is None:
        with_bias = bool(qkv_b.any() or proj_b.any())

    wq, wk, wv = qkv_w[0:C], qkv_w[C:2 * C], qkv_w[2 * C:3 * C]
    bq, bk, bv = qkv_b[0:C], qkv_b[C:2 * C], qkv_b[2 * C:3 * C]
    # de-interleave RoPE pairs within each head: even d first, then odd d
    dperm = np.concatenate([np.arange(0, D, 2), np.arange(1, D, 2)])
    hperm = (np.arange(H)[:, None] * D + dperm[None, :]).reshape(-1)  # [C]
    wk = wk[hperm]
    wq = wq[hperm]
    bk = bk[hperm]
    bq = bq[hperm]
    wcat_flat = np.concatenate([wk, wq], axis=0).T  # [C, 2C]
    wvT = wv.T  # [C, C]
    vcat = np.ascontiguousarray(
        wvT.reshape(CCH, 128, 2, 512).transpose(2, 1, 0, 3)
        .reshape(2, 128, 4 * C)
    ).astype(ml_dtypes.bfloat16)
    # pre-chunk to SBUF layout: slab cb -> [p, kc*128 + r] = wcat[kc*128+p, cb*128+r]
    wcat = np.ascontiguousarray(
        wcat_flat.reshape(CCH, 128, 2 * C // 128, 128).transpose(2, 1, 0, 3)
        .reshape(2 * C // 128, 128, C)
    ).astype(ml_dtypes.bfloat16)
    pTb = np.ascontiguousarray(proj_w.T).astype(ml_dtypes.bfloat16)
    # cos/sin rows in the de-interleaved order; sin carries the rotate sign
    cosT = np.tile(cos_emb.T[dperm], (2, 1))   # [128, L]
    sgn = np.concatenate([-np.ones(D // 2), np.ones(D // 2)])[:, None]
    sinT = np.tile(sin_emb.T[dperm] * sgn, (2, 1))

    in_maps = []
    for core in range(8):
        b, half = core // 2, core % 2
        q0 = half * LQ
        idx = np.concatenate(
            [np.arange(q0, q0 + LQ), np.arange(0, q0), np.arange(q0 + LQ, L)]
        )
        xT_p = np.ascontiguousarray(x[b].T[:, idx]).astype(ml_dtypes.bfloat16)
        m = dict(
            xT=xT_p,
            ones64=np.ones((1, 64), ml_dtypes.bfloat16),
            vcat=vcat,
            wcat=wcat, pT=pTb,
            cosP=np.ascontiguousarray(cosT[:, idx]).astype(ml_dtypes.bfloat16),
            sinP=np.ascontiguousarray(sinT[:, idx]).astype(ml_dtypes.bfloat16),
        )
        if with_bias:
            m["bK"] = np.ascontiguousarray(bk.reshape(NPAIR, 128).T)
            m["bQ"] = np.ascontiguousarray(bq.reshape(NPAIR, 128).T)
            m["vb"] = np.ascontiguousarray(np.tile(bv[None, :], (128, 1)))
            m["pb"] = np.ascontiguousarray(
                np.tile(proj_b[None, :], (128, 1))).astype(f32)
        in_maps.append(m)
    return in_maps


def kernel(x, cos_emb, sin_emb, qkv_w, qkv_b, proj_w, proj_b):
    from concourse.bass_utils import run_bass_kernel_spmd

    with_bias = bool(np.asarray(qkv_b).any() or np.asarray(proj_b).any())
    in_maps = make_in_maps(x, cos_emb, sin_emb, qkv_w, qkv_b, proj_w, proj_b,
                           with_bias=with_bias)
    res = run_bass_kernel_spmd(_get_nc(with_bias), in_maps,
                               core_ids=list(range(8)))
    out = np.empty((B, L, C), np.float32)
    for core in range(8):
        b, half = core // 2, core % 2
        out[b, half * LQ:(half + 1) * LQ, :] = res.results[core]["out"]
    return out


# revision 24
# speedup vs baseline: 1.1925x; 1.1925x over previous
"""Trainium2 Bass kernel: self-attention block with interleaved RoPE.

Reference computation (per batch b):
    qkv = x @ qkv_w.T + qkv_b            # [L, 3C]
    q,k,v per head (H=16, D=64); q,k get interleaved RoPE
    attn = softmax(scale * q @ k.T) @ v  # per head
    out  = concat_heads(attn) @ proj_w.T + proj_b

Sharding: 8 cores = 4 batches x 2 query-halves. Each core computes K,V for
the full sequence of its batch (duplicated across the half-pair) and Q for
its own 1024 queries -> disjoint output slices, no collectives.

Per-core layout trick: the host permutes the L axis of x^T (and cos/sin) so
this core's queries are always columns 0:1024 -> one SPMD graph for all
cores. Key order is permuted consistently everywhere (softmax is
order-invariant over keys).

On-chip dataflow (all "transposed" so no PE transposes are needed):
  K'^T/Q'^T [d, L] tiles  <- matmul(lhsT=W^T slab, rhs=x^T) + RoPE on DVE
     (rotate_half realized as 32-partition block-swap DMAs)
  S^T [k,q] = matmul(lhsT=K'^T, rhs=Q'^T) for BOTH heads of a pair,
     row-group packed (sel=0 -> array rows 0:64, sel=1 -> rows 64:128,
     auto tile_position) -> the two score MMs run CONCURRENTLY.
  P^T = exp(scale*S^T) on ACT (no max subtraction: scores ~N(0,1), exp
     cannot overflow in f32)
  AV^T [65,q] = matmul(lhsT=[V | ones], rhs=P^T)   row 64 = softmax denom
  normalize: PSUM evacuate on ACT (Copy shares the exp table set), then
     the [1,1024] denominator row is DMA-reshaped to [16,64] so the DVE
     reciprocal isn't lane-starved, reshaped+cast back to bf16 by one
     gpsimd software-DGE DMA, broadcast with a cheap bf16 ones-matmul,
     and multiplied into stk on DVE.
  proj: stacked head pairs -> matmul over 8 contraction chunks; PSUM->SBUF
     evacuation adds the (pre-replicated) bias when biases are nonzero.

Schedule: ONE flat stream of 256 attention (pair,j,t) steps paced by ACT
(exp ~1.0us/step). ALL generation work (K/Q/V matmul chunks, split into
4-MM half-units ~0.86us each) sits in a deadline-sorted queue; ~one unit is
popped per attention step so the PE never idles (HAM stays at K=8/8) and
there is no un-overlapped gen prologue. need() force-pops units whose
output an attention step is about to consume. Normalize finishers and proj
tiles enter the same queue once their inputs exist; the final tail
interleaves the last normalize chain with proj head-chunks.

The common case (all-zero qkv/proj biases, as in this problem's inputs)
builds a graph with no bias adds at all and 1.5MB less prologue DMA; a
with-bias graph is built instead if kernel() sees nonzero biases.
"""

import numpy as np
import ml_dtypes

import concourse.bass as bass
import concourse.mybir as mybir
from concourse.tile import TileContext

F32 = mybir.dt.float32
BF16 = mybir.dt.bfloat16
AOP = mybir.AluOpType
AFT = mybir.ActivationFunctionType

B, L, C = 4, 2048, 1024
H, D = 16, 64
LQ = L // 2            # queries per core
NPAIR = H // 2         # 8 head pairs
NG = 4                 # head groups
GH = H // NG           # 4 heads per group
GP = GH // 2           # 2 pairs per group
CCH = C // 128         # 8 contraction chunks
LT = L // 128          # 16 key tiles
QT = LQ // 128         # 8 query row-tiles
SCALE = float(D) ** -0.5


def build_nc(with_bias=False):
    nc = bass.Bass()
    xT = nc.declare_dram_parameter("xT", [C, L], BF16, isOutput=False)
    wcat = nc.declare_dram_parameter("wcat", [2 * C // 128, 128, C], BF16, isOutput=False)
    pT = nc.declare_dram_parameter("pT", [C, C], BF16, isOutput=False)
    cosP = nc.declare_dram_parameter("cosP", [128, L], BF16, isOutput=False)
    sinP = nc.declare_dram_parameter("sinP", [128, L], BF16, isOutput=False)
    if with_bias:
        pb = nc.declare_dram_parameter("pb", [128, C], F32, isOutput=False)
        bK = nc.declare_dram_parameter("bK", [128, NPAIR], F32, isOutput=False)
        bQ = nc.declare_dram_parameter("bQ", [128, NPAIR], F32, isOutput=False)
        vb = nc.declare_dram_parameter("vb", [128, C], F32, isOutput=False)
    ones64 = nc.declare_dram_parameter("ones64", [1, 64], BF16, isOutput=False)
    vcat = nc.declare_dram_parameter("vcat", [2, 128, 4 * C], BF16, isOutput=False)
    out = nc.declare_dram_parameter("out", [LQ, C], F32, isOutput=True)

    with TileContext(nc) as tc:
        with (
            tc.tile_pool(name="persist", bufs=1) as P1,
            tc.tile_pool(name="wpool", bufs=2) as WP,
            tc.tile_pool(name="vwpool", bufs=2) as VW,
            tc.tile_pool(name="ktpool", bufs=2) as KTP,
            tc.tile_pool(name="work", bufs=2) as WK,
            tc.tile_pool(name="osbpool", bufs=4) as OSB,
            tc.tile_pool(name="espool", bufs=3) as ESP,
            tc.tile_pool(name="vtpool", bufs=2) as VTP,
            tc.tile_pool(name="pgen", bufs=2, space="PSUM") as PGEN,
            tc.tile_pool(name="pss", bufs=2, space="PSUM") as PSS,
            tc.tile_pool(name="pav", bufs=1, space="PSUM") as PAV,
        ):
            # ---- PE warmup: ~4us of tiny matmuls on a memset tile so the
            # HAM clock-gate reaches K=8/8 while the first DMAs land.
            # (Extending to 90 MMs measured MUCH slower — don't.)
            wrm = P1.tile([128, 64], BF16, name="wrm", tag="wrm")
            nc.vector.memset(wrm[:, :], 0.125)
            pswm = PGEN.tile([64, 64], F32, name="pswm", tag="gen")
            for _ in range(48):
                nc.tensor.matmul(pswm[:, :], wrm[:, 0:64], wrm[:, 0:64],
                                 start=True, stop=True)

            # ---- x^T: first quarter split in half (the very first K/Q gen
            # chunk only needs contraction rows 0:512), rest by quarters.
            xtile = P1.tile([128, CCH * L], BF16, name="xtile", tag="xtile")
            xt = [xtile[:, i * L:(i + 1) * L] for i in range(CCH)]
            xv3 = xtile.rearrange("p (k l) -> p k l", l=L)
            xs3 = xT.rearrange("(k p) l -> p k l", p=128)
            nc.sync.dma_start(out=xv3[:, 0:4, 0:512], in_=xs3[:, 0:4, 0:512])
            nc.sync.dma_start(out=xv3[:, 4:8, 0:512], in_=xs3[:, 4:8, 0:512])
            for lh in range(1, 4):
                lsl = slice(lh * (L // 4), (lh + 1) * (L // 4))
                nc.sync.dma_start(out=xv3[:, :, lsl], in_=xs3[:, :, lsl])
            touch_n = [0]

            def touch(t):
                # tiny DVE read so later DVE ops don't each carry this
                # tile's DMA-queue semaphore wait (walrus wait-count limit)
                sc = P1.tile([1, 1], F32, name=f"scr{touch_n[0]}",
                             tag=f"scr{touch_n[0]}")
                touch_n[0] += 1
                nc.vector.tensor_copy(sc[0:1, 0:1], t[0:1, 0:1])

            # persistent small loads on the SCALAR engine's DMA queue so
            # they don't serialize behind the 4MB x load on sync
            cos_sb = P1.tile([128, L], BF16, name="cos_sb", tag="cos_sb")
            nc.scalar.dma_start(out=cos_sb[:, :], in_=cosP[:, :])
            touch(cos_sb)
            sin_sb = P1.tile([128, L], BF16, name="sin_sb", tag="sin_sb")
            nc.scalar.dma_start(out=sin_sb[:, :], in_=sinP[:, :])
            touch(sin_sb)
            bias_sb = {}
            if with_bias:
                for nm, prm in (("bK", bK), ("bQ", bQ)):
                    t = P1.tile([128, NPAIR], F32, name=f"{nm}_sb",
                                tag=f"{nm}_sb")
                    nc.scalar.dma_start(out=t[:, :], in_=prm[:, :])
                    touch(t)
                    bias_sb[nm] = t
                vb_sb = P1.tile([128, C], F32, name="vb_sb", tag="vb_sb")
                nc.scalar.dma_start(out=vb_sb[:, :], in_=vb[:, :])
                touch(vb_sb)
            ones_sb = P1.tile([1, 64], BF16, name="ones_sb", tag="ones_sb")
            nc.scalar.dma_start(out=ones_sb[:, :], in_=ones64[:, :])
            # proj weights (and bias) are needed only from step ~190 on;
            # loaded via deferred queue units to keep prologue DMA light.
            pt = [P1.tile([128, C], BF16, name=f"pt{i}", tag=f"pt{i}")
                  for i in range(CCH)]
            pb_sb = (P1.tile([128, C], F32, name="pb_sb", tag="pb_sb")
                     if with_bias else None)
            stk = []
            for i in range(NPAIR):
                stk.append(P1.tile([128, LQ], BF16, name=f"stk{i}", tag=f"stk{i}"))

            kt_of = {}   # pair -> ktile
            qt_of = {}   # pair -> qtile
            wt_of = {}   # (pair, kind) -> weight slab
            vsl_of = {}  # sg -> V weight slab
            vt_sg = {}   # sg -> list of vt tiles
            psm_of = {}  # (pair, kind, j) -> open gen psum
            psv_of = {}  # (sg, t) -> open v psum
            pj_of = {}   # (qi, jn) -> open proj psum

            # ---------- work units ----------
            def mk_slab(p):
                def f():
                    lp = p % GP
                    kt_of[p] = KTP.tile([128, L], BF16, name=f"kt{lp}",
                                        tag=f"ktile{lp}")
                    qt_of[p] = KTP.tile([128, LQ], BF16, name=f"qt{lp}",
                                        tag=f"qtile{lp}")
                    for kind, base in ((0, 0), (1, C)):
                        wt_ = WP.tile([128, C], BF16, name="wt_",
                                      tag=f"wt{lp}_{kind}")
                        nc.gpsimd.dma_start(
                            out=wt_[:, :], in_=wcat[base // 128 + p])
                        wt_of[(p, kind)] = wt_
                return f

            def mk_vsl(sg):
                def f():
                    vt_sg[sg] = [None] * LT
                    vsl = VW.tile([128, 4 * C], BF16, name="vsl", tag="vsl")
                    nc.gpsimd.dma_start(out=vsl[:, :], in_=vcat[sg])
                    vsl_of[sg] = vsl
                return f

            def mk_ptload(i):
                def f():
                    nc.sync.dma_start(out=pt[i][:, :],
                                      in_=pT[i * 128:(i + 1) * 128, :])
                    if with_bias and i == 0:
                        nc.scalar.dma_start(out=pb_sb[:, :], in_=pb[:, :])
                return f

            def mk_kq(p, kind, j, h):
                def f():
                    key = (p, kind, j)
                    if h == 0:
                        psm_of[key] = PGEN.tile([128, 512], F32, name="psm",
                                                tag="gen")
                    psm = psm_of[key]
                    wt_ = wt_of[(p, kind)]
                    jsl = slice(j * 512, (j + 1) * 512)
                    for kc in range(4 * h, 4 * h + 4):
                        nc.tensor.matmul(
                            psm[:, :],
                            wt_[:, kc * 128:(kc + 1) * 128],
                            xt[kc][:, jsl],
                            start=(kc == 0), stop=(kc == CCH - 1),
                        )
                    if h == 0:
                        return
                    # RoPE: raw (biased) values in bf16, rotate-half as
                    # 32-partition block-swap DMAs (host de-interleaved the
                    # d order: evens then odds; sinP carries the sign)
                    dst = kt_of[p] if kind == 0 else qt_of[p]
                    kr = WK.tile([128, 512], BF16, name="kr", tag="kr")
                    if with_bias:
                        bmain = "bK" if kind == 0 else "bQ"
                        nc.vector.tensor_scalar_add(
                            kr[:, :], psm[:, :], bias_sb[bmain][:, p:p + 1])
                    else:
                        nc.vector.tensor_copy(kr[:, :], psm[:, :])
                    krs = WK.tile([128, 512], BF16, name="krs", tag="krs")
                    for bb in range(4):
                        so = (bb ^ 1) * 32
                        nc.sync.dma_start(
                            out=krs[bb * 32:(bb + 1) * 32, :],
                            in_=kr[so:so + 32, :],
                        )
                    if with_bias:
                        nc.vector.scalar_tensor_tensor(
                            out=dst[:, jsl], in0=psm[:, :],
                            scalar=bias_sb[bmain][:, p:p + 1],
                            in1=cos_sb[:, jsl], op0=AOP.add, op1=AOP.mult,
                        )
                    else:
                        nc.vector.tensor_tensor(
                            out=dst[:, jsl], in0=psm[:, :],
                            in1=cos_sb[:, jsl], op=AOP.mult,
                        )
                    tmp = WK.tile([128, 512], BF16, name="tmp", tag="tmp")
                    nc.vector.tensor_tensor(
                        out=tmp[:, :], in0=krs[:, :], in1=sin_sb[:, jsl],
                        op=AOP.mult,
                    )
                    nc.vector.tensor_tensor(
                        out=dst[:, jsl], in0=dst[:, jsl], in1=tmp[:, :],
                        op=AOP.add,
                    )
                return f

            def mk_v(sg, t, h):
                def f():
                    key = (sg, t)
                    if h == 0:
                        vt_sg[sg][t] = VTP.tile([128, 2 * GH * 65], BF16,
                                                name="vt", tag=f"vt{t}")
                        psv_of[key] = PGEN.tile([128, 512], F32, name="psv",
                                                tag="gen")
                    psv = psv_of[key]
                    vsl = vsl_of[sg]
                    for kc in range(4 * h, 4 * h + 4):
                        nc.tensor.matmul(
                            psv[:, :],
                            xt[kc][:, t * 128:(t + 1) * 128],
                            vsl[:, kc * 512:(kc + 1) * 512],
                            start=(kc == 0), stop=(kc == CCH - 1),
                        )
                    if h == 0:
                        return
                    vt = vt_sg[sg][t]
                    vt3 = vt.rearrange("p (a s) -> p a s", s=65)
                    if with_bias:
                        nc.vector.tensor_tensor(
                            out=vt3[:, :, 0:64],
                            in0=psv[:, :].rearrange("p (a d) -> p a d", d=64),
                            in1=vb_sb[:, sg * 512:(sg + 1) * 512]
                            .rearrange("p (a d) -> p a d", d=64),
                            op=AOP.add,
                        )
                    else:
                        # evacuate on ACT (Copy, same table set as Exp):
                        # keeps the DVE free of psv-consumer backlog that
                        # otherwise stalls gen-psum buffer reuse on the PE
                        nc.scalar.copy(
                            vt3[:, :, 0:64],
                            psv[:, :].rearrange("p (a d) -> p a d", d=64),
                        )
                    nc.vector.memset(vt3[:, :, 64:65], 1.0)
                return f

            def mk_proj(qi, jn, h, ptag="gen"):
                # h0: contraction chunks 0..5 (stk of pairs 0..5, available
                # well before the tail); h1: chunks 6,7 + evacuate. At the
                # tail, half the tiles borrow the (by then idle) ss psum
                # bufs so 4 accumulations can be open at once.
                def f():
                    key = (qi, jn)
                    if h == 0:
                        pool = PGEN if ptag == "gen" else PSS
                        pj_of[key] = pool.tile([128, 512], F32, name="pj",
                                               tag=ptag)
                    pj = pj_of[key]
                    rng = range(0, 6) if h == 0 else range(6, CCH)
                    for cp in rng:
                        nc.tensor.matmul(
                            pj[:, :],
                            stk[cp][:, qi * 128:(qi + 1) * 128],
                            pt[cp][:, jn * 512:(jn + 1) * 512],
                            start=(cp == 0), stop=(cp == CCH - 1),
                        )
                    if h == 0:
                        return
                    osl = OSB.tile([128, 512], F32, name="osb", tag="osb")
                    if with_bias:
                        nc.vector.tensor_tensor(
                            out=osl[:, :], in0=pj[:, :],
                            in1=pb_sb[:, jn * 512:(jn + 1) * 512], op=AOP.add,
                        )
                    else:
                        nc.vector.tensor_copy(osl[:, :], pj[:, :])
                    nc.sync.dma_start(
                        out=out[qi * 128:(qi + 1) * 128,
                                jn * 512:(jn + 1) * 512],
                        in_=osl[:, :],
                    )
                return f

            # ---------- queue: (label, light, is_h0, fn) ----------
            queue = []
            emitted = set()

            def build_units():
                units = []  # (deadline, seq, label, light, is_h0, fn)
                seq = [0]

                def add(dl, label, light, is_h0, fn):
                    units.append((dl, seq[0], label, light, is_h0, fn))
                    seq[0] += 1

                for p in range(8):
                    base = 32 * p
                    add(base - 6, f"slab{p}", True, False, mk_slab(p))
                    if p % 4 == 0:
                        sg = p // 4
                        add(base - 5.5, f"vsl{sg}", True, False, mk_vsl(sg))
                    # K/Q units get ~4 steps of slack before first use: the
                    # RoPE tail (kr copy -> 4 swap DMAs -> 2 DVE ops) is
                    # ~5us of latency that otherwise blocks the pair's
                    # first score matmul at the boundary
                    for jk in range(4):
                        for h in range(2):
                            add(base + 4 * jk - 4.5 + 0.01 * h,
                                f"K{p}_{jk}" if h else f"K{p}_{jk}h0",
                                False, h == 0, mk_kq(p, 0, jk, h))
                    for jq in range(2):
                        for h in range(2):
                            add(base + 16 * jq - 4.4 + 0.01 * h,
                                f"Q{p}_{jq}" if h else f"Q{p}_{jq}h0",
                                False, h == 0, mk_kq(p, 1, jq, h))
                for sg in range(2):
                    for t in range(LT):
                        for h in range(2):
                            add(128 * sg + t - 2.3 + 0.01 * h,
                                f"V{sg}_{t}" if h else f"V{sg}_{t}h0",
                                False, h == 0, mk_v(sg, t, h))
                for i in range(CCH):
                    add(150 + 2 * i, f"pt{i}", True, False, mk_ptload(i))
                units.sort(key=lambda u: (u[0], u[1]))
                return [(lbl, light, h0, fn) for _, _, lbl, light, h0, fn in units]

            queue.extend(build_units())

            def pop_one():
                lbl, light, h0, fn = queue.pop(0)
                fn()
                emitted.add(lbl)
                return light

            def pop_some():
                while queue:
                    if not pop_one():
                        break

            def need(lbl):
                while lbl not in emitted:
                    assert queue, f"need({lbl}) but queue empty"
                    pop_one()

            def insert_unit(unit, min_pos):
                # insert at the first group boundary (is_h0 or light) at or
                # after min_pos so an h0/h1 psum pair is never split
                pos = min(min_pos, len(queue))
                while pos < len(queue) and not (queue[pos][1] or queue[pos][2]):
                    pos += 1
                queue.insert(pos, unit)

            # ---------- normalize finisher ----------
            def fin_pre(p, j, av, tail=False):
                # PSUM->SBUF evacuation on ACT (Copy shares the exp table
                # set, no reload): frees the single av bank ~1.2us after the
                # last AV matmul regardless of DVE backlog, so the next
                # (pair,j)'s AV t0 doesn't stall the PE.
                avc = WK.tile([65, 1024], F32, name="avc", tag="avc")
                nc.scalar.copy(avc[:, :], av[:, :])
                # reciprocal of the [1,1024] denominator row is DVE
                # lane-starved (6.6us). DMA-reshape to [16,64] (64 elems
                # per lane), reciprocal there (~0.5us), then one gpsimd
                # software-DGE DMA reshapes back AND casts f32->bf16.
                # (Routing these DMAs via the sync queue measured slower:
                # they contend with the rotate-half swap DMAs there.)
                rds = WK.tile([16, 64], F32, name="rds", tag="rds")
                rdr = WK.tile([16, 64], F32, name="rdr", tag="rdr")
                rdb = WK.tile([1, 1024], BF16, name="rdb", tag="rdb")
                if tail:
                    # latency matters only here: hardware-DGE DMAs on the
                    # (idle at tail) scalar queue + a cheap [16,64] DVE cast
                    nc.scalar.dma_start(out=rds[:, :], in_=avc[64:65, :])
                    nc.vector.reciprocal(rdr[:, :], rds[:, :])
                    rdc = WK.tile([16, 64], BF16, name="rdc", tag="rdc")
                    nc.vector.tensor_copy(rdc[:, :], rdr[:, :])
                    nc.scalar.dma_start(out=rdb[0:1, :], in_=rdc[:, :])
                else:
                    nc.gpsimd.dma_start(out=rds[:, :], in_=avc[64:65, :])
                    nc.vector.reciprocal(rdr[:, :], rds[:, :])
                    nc.gpsimd.dma_start(out=rdb[0:1, :], in_=rdr[:, :])

                jsl = slice(j * 512, (j + 1) * 512)

                def post():
                    # the tail finisher sits between two OPEN proj psum
                    # accumulations; allocating from the (now idle) PAV
                    # bank avoids a "gen"-tag reuse cycle there.
                    if tail:
                        bcw = PAV.tile([64, 1024], F32, name="bcw", tag="av")
                        bcs = [bcw[:, 0:512], bcw[:, 512:1024]]
                    else:
                        bcs = [PGEN.tile([64, 512], F32, name="bc", tag="gen")
                               for _ in range(2)]
                    for sel in range(2):
                        bc = bcs[sel]
                        nc.tensor.matmul(
                            bc[:, :], ones_sb[:, :],
                            rdb[0:1, sel * 512:(sel + 1) * 512],
                            start=True, stop=True,
                        )
                        nc.vector.tensor_tensor(
                            out=stk[p][sel * 64:(sel + 1) * 64, jsl],
                            in0=avc[0:64, sel * 512:(sel + 1) * 512],
                            in1=bc[:, :], op=AOP.mult,
                        )
                return post

            # ---------- main schedule ----------
            pj_order = []
            for g in range(NG - 1):
                for lp in range(GP):
                    for j in range(2):
                        pj_order.append((2 * g + lp, j))
            for j in range(2):
                for lp in range(GP):
                    pj_order.append((6 + lp, j))

            def pop_allowed(p, j, t):
                if (p, j) in ((6, 1), (7, 1)):
                    return 5 <= t <= 14 and t != 10
                return t not in (0, 5, 10, 15)

            for p, j in pj_order:
                sg = p // 4
                voff = ((p // 2) % 2) * GH * 65
                lp = p % GP
                jsl = slice(j * 512, (j + 1) * 512)
                need(f"Q{p}_{j}")
                av = PAV.tile([65, 1024], F32, name="av", tag="av")
                for t in range(LT):
                    need(f"K{p}_{t // 4}")
                    need(f"V{sg}_{t}")
                    if pop_allowed(p, j, t) and queue:
                        pop_some()
                    ss = PSS.tile([128, 1024], F32, name="ss", tag="ss")
                    for sel in range(2):
                        nc.tensor.matmul(
                            ss[:, sel * 512:(sel + 1) * 512],
                            kt_of[p][sel * 64:(sel + 1) * 64,
                                     t * 128:(t + 1) * 128],
                            qt_of[p][sel * 64:(sel + 1) * 64, jsl],
                            start=True, stop=True,
                        )
                    es = ESP.tile([128, 1024], BF16, name="es", tag="es")
                    nc.scalar.activation(es[:, :], ss[:, :], AFT.Exp,
                                         scale=SCALE)
                    for sel in range(2):
                        lh = lp * 2 + sel
                        nc.tensor.matmul(
                            av[0:65, sel * 512:(sel + 1) * 512],
                            vt_sg[sg][t][:, voff + lh * 65:voff + (lh + 1) * 65],
                            es[:, sel * 512:(sel + 1) * 512],
                            start=(t == 0), stop=(t == LT - 1),
                        )
                post = fin_pre(p, j, av, tail=(p, j) == (7, 1))
                if (p, j) == (7, 1):
                    # tail: FOUR open proj accumulations (alternating
                    # gen/ss psum tags) cover the last normalize chain,
                    # then the finisher, then h1 units (which read
                    # stk[6],stk[7]) pipelined with the remaining h0s
                    tiles = [(qi, jn) for qi in range(QT // 2, QT)
                             for jn in range(2)]
                    tag = {t: ("gen" if i % 2 == 0 else "ss")
                           for i, t in enumerate(tiles)}
                    hu = {(qi, jn, h): (f"pj{qi}_{jn}h{h}", False, h == 0,
                                        mk_proj(qi, jn, h, tag[(qi, jn)]))
                          for qi, jn in tiles for h in range(2)}
                    for i in range(4):
                        queue.append(hu[tiles[i] + (0,)])
                    queue.append((f"fin{p}_{j}", True, False, post))
                    for i, (qi, jn) in enumerate(tiles):
                        queue.append(hu[(qi, jn, 1)])
                        if i + 4 < len(tiles):
                            queue.append(hu[tiles[i + 4] + (0,)])
                else:
                    insert_unit((f"fin{p}_{j}", True, False, post), 3)
                if (p, j) == (7, 0):
                    for qi in range(QT // 2):
                        for jn in range(2):
                            for h in range(2):
                                queue.append((f"pj{qi}_{jn}h{h}", False,
                                              h == 0, mk_proj(qi, jn, h)))
            while queue:
                pop_one()
    return nc


_CACHE = {}

# walrus in this toolchain enforces small per-instruction sync-wait budgets
# (DMACopy/TensorCopy: 1, most compute: 2). Tile emits more on a few
# instructions, so split the excess into standalone EventSemaphore
# wait-carriers on the same engine (the raw-bass wait_ge pattern).
_WAIT_BUDGET = {"DMACopy": 1, "TensorCopy": 1, "Reciprocal": 1, "Memset": 1,
                "Iota": 1, "FindIndex8": 1}
_DEFAULT_BUDGET = 1


def _split_waits(bir_bytes):
    import json
    bir = json.loads(bir_bytes)
    ctr = 0
    for fn in bir["functions"]:
        for blk in fn["blocks"]:
            insts = blk.get("instructions")
            if not insts:
                continue
            out = []
            for inst in insts:
                si = inst.get("sync_info")
                if si and si.get("on_wait"):
                    waits = si["on_wait"]
                    b = _WAIT_BUDGET.get(inst.get("opcode"), _DEFAULT_BUDGET)
                    if len(waits) > b:
                        excess, keep = waits[:-b], waits[-b:]
                        for w in excess:
                            ctr += 1
                            out.append({
                                "debug": inst.get("debug", 0),
                                "engine": inst["engine"],
                                "ins": [], "outs": [],
                                "name": f"wfix{ctr}",
                                "opcode": "EventSemaphore",
                                "sync_info": {"on_update": [], "on_wait": [w]},
                            })
                        si["on_wait"] = keep
                out.append(inst)
            blk["instructions"] = out
    return json.dumps(bir).encode()


def _get_nc(with_bias=False):
    key = f"nc{int(with_bias)}"
    if key not in _CACHE:
        nc = build_nc(with_bias)
        fixed = _split_waits(nc.to_json_bytes())
        nc.to_json_bytes = lambda fixed=fixed: fixed
        _CACHE[key] = nc
    return _CACHE[key]


def make_in_maps(x, cos_emb, sin_emb, qkv_w, qkv_b, proj_w, proj_b,
                 with_bias=None):
    f32 = np.float32
    x = np.asarray(x, f32)
    qkv_w = np.asarray(qkv_w, f32)
    qkv_b = np.asarray(qkv_b, f32)
    proj_w = np.asarray(proj_w, f32)
    proj_b = np.asarray(proj_b, f32)
    cos_emb = np.asarray(cos_emb, f32)
    sin_emb = np.asarray(sin_emb, f32)
    if with_bias is None:
        with_bias = bool(qkv_b.any() or proj_b.any())

    wq, wk, wv = qkv_w[0:C], qkv_w[C:2 * C], qkv_w[2 * C:3 * C]
    bq, bk, bv = qkv_b[0:C], qkv_b[C:2 * C], qkv_b[2 * C:3 * C]
    # de-interleave RoPE pairs within each head: even d first, then odd d
    dperm = np.concatenate([np.arange(0, D, 2), np.arange(1, D, 2)])
    hperm = (np.arange(H)[:, None] * D + dperm[None, :]).reshape(-1)  # [C]
    wk = wk[hperm]
    wq = wq[hperm]
    bk = bk[hperm]
    bq = bq[hperm]
    wcat_flat = np.concatenate([wk, wq], axis=0).T  # [C, 2C]
    wvT = wv.T  # [C, C]
    vcat = np.ascontiguousarray(
        wvT.reshape(CCH, 128, 2, 512).transpose(2, 1, 0, 3)
        .reshape(2, 128, 4 * C)
    ).astype(ml_dtypes.bfloat16)
    # pre-chunk to SBUF layout: slab cb -> [p, kc*128 + r] = wcat[kc*128+p, cb*128+r]
    wcat = np.ascontiguousarray(
        wcat_flat.reshape(CCH, 128, 2 * C // 128, 128).transpose(2, 1, 0, 3)
        .reshape(2 * C // 128, 128, C)
    ).astype(ml_dtypes.bfloat16)
    pTb = np.ascontiguousarray(proj_w.T).astype(ml_dtypes.bfloat16)
    # cos/sin rows in the de-interleaved order; sin carries the rotate sign
    cosT = np.tile(cos_emb.T[dperm], (2, 1))   # [128, L]
    sgn = np.concatenate([-np.ones(D // 2), np.ones(D // 2)])[:, None]
    sinT = np.tile(sin_emb.T[dperm] * sgn, (2, 1))

    in_maps = []
    for core in range(8):
        b, half = core // 2, core % 2
        q0 = half * LQ
        idx = np.concatenate(
            [np.arange(q0, q0 + LQ), np.arange(0, q0), np.arange(q0 + LQ, L)]
        )
        xT_p = np.ascontiguousarray(x[b].T[:, idx]).astype(ml_dtypes.bfloat16)
        m = dict(
            xT=xT_p,
            ones64=np.ones((1, 64), ml_dtypes.bfloat16),
            vcat=vcat,
            wcat=wcat, pT=pTb,
            cosP=np.ascontiguousarray(cosT[:, idx]).astype(ml_dtypes.bfloat16),
            sinP=np.ascontiguousarray(sinT[:, idx]).astype(ml_dtypes.bfloat16),
        )
        if with_bias:
            m["bK"] = np.ascontiguousarray(bk.reshape(NPAIR, 128).T)
            m["bQ"] = np.ascontiguousarray(bq.reshape(NPAIR, 128).T)
            m["vb"] = np.ascontiguousarray(np.tile(bv[None, :], (128, 1)))
            m["pb"] = np.ascontiguousarray(
                np.tile(proj_b[None, :], (128, 1))).astype(f32)
        in_maps.append(m)
    return in_maps


def kernel(x, cos_emb, sin_emb, qkv_w, qkv_b, proj_w, proj_b):
    from concourse.bass_utils import run_bass_kernel_spmd

    with_bias = bool(np.asarray(qkv_b).any() or np.asarray(proj_b).any())
    in_maps = make_in_maps(x, cos_emb, sin_emb, qkv_w, qkv_b, proj_w, proj_b,
                           with_bias=with_bias)
    res = run_bass_kernel_spmd(_get_nc(with_bias), in_maps,
                               core_ids=list(range(8)))
    out = np.empty((B, L, C), np.float32)
    for core in range(8):
        b, half = core // 2, core % 2
        out[b, half * LQ:(half + 1) * LQ, :] = res.results[core]["out"]
    return out
